# revision 30
# baseline (speedup 1.0000x reference)
"""VQ codebook-lookup kernel v2 for trn2 (8 NeuronCores, SPMD data-parallel).

For x: [32, 64, 64, 64] (BCHW) and codebook: [1024, 64]:
    out[b,:,h,w] = codebook[argmin_k fl(fl(A+b_k) - c_k)],  c_k = fl(2 flat.e_k)
matching the f32 reference's rounding/tie structure.

Key algebraic reduction vs the old kernel: since fl is monotone,
    argmin_k fl(T_t - c_k)  ==  first k maximizing round(c_k / ulp(T_t))
so instead of reproducing fl(A+b) on the PE (6-row chain, 4 matmul passes),
the host pre-scales x by s_t = 1/ulp(T_t) (a power of 2, exact) and the PE
computes y = c*s + 8192(.5) in TWO bf16-split passes:
    m_a = [xl;xh] . [e2h;e2l]        (contraction 128: xl*e2h + xh*e2l)
    m_b = [xh;1;1] . [e2h;8192;HALF] (contraction 66: xh*e2h + bias)
A tensor_tensor_scan(max) reads the PSUM, tracks the running max in f32,
and its int16 downcast quantizes to the reference's rounding grid.  The
first-argmax index (reference tie-break) is the count of (r_k < m), one
4x-mode DVE tensor_scalar(is_lt, accum=add).  Scans are split DVE/gpsimd.

Sharding: batch-parallel, core i handles batches [4i, 4i+4).
Output = gathered codebook rows (STE forward value == q up to ~2e-4 rel).
"""

import sys
import numpy as np
import ml_dtypes
from contextlib import ExitStack

for p in ("/opt/trn_rl_repo",):
    if p not in sys.path:
        sys.path.append(p)

import concourse.bacc as bacc
import concourse.mybir as mybir
import concourse.tile as tile
from concourse import bass_utils, library_config

F32 = mybir.dt.float32
BF16 = mybir.dt.bfloat16
I16 = mybir.dt.int16

B, D, H, W = 32, 64, 64, 64
K = 1024
NCORES = 8
BPC = B // NCORES          # batches per core = 4
TOK = H * W                # tokens per batch = 4096
NTILE = TOK // 128         # 128-token tiles per batch = 32

# knobs
HALF = 0.0                 # 0.5 if the f32->i16 downcast truncates, 0.0 if RNE
EMIN_ROW, EMAX_ROW = 4, 9  # bq indicator row exponent range (6 rows)
SCAN_GP_32 = 18            # tile's scan runs on gpsimd when (tidx*this)%32 < this
COUNT_ACT_16 = 4           # q8-group's counts run on ACT when (qidx % 16) < this
TAIL_DVE = 0               # last N tiles force DVE scans (shorter drain)

_cache = {}


def _bf16(v):
    return v.astype(ml_dtypes.bfloat16)


def _split2(v):
    h = _bf16(v)
    l = _bf16(v - h.astype(np.float32))
    return h, l


def _build_module():
    nc = bacc.Bacc("TRN2", target_bir_lowering=False, debug=False, num_devices=NCORES)

    d_xma = nc.dram_tensor("xma", [BPC, 128, TOK], BF16, kind="ExternalInput").ap()
    d_xmb = nc.dram_tensor("xmb", [BPC, 70, TOK], BF16, kind="ExternalInput").ap()
    d_ema = nc.dram_tensor("ema", [128, K], BF16, kind="ExternalInput").ap()
    d_emb = nc.dram_tensor("emb", [70, K], BF16, kind="ExternalInput").ap()
    d_cbt = nc.dram_tensor("cbt", [128, K], F32, kind="ExternalInput").ap()
    d_out = nc.dram_tensor("out", [2, 128, TOK], F32, kind="ExternalOutput").ap()

    with tile.TileContext(nc) as tc, ExitStack() as ctx:
        sb = ctx.enter_context(tc.tile_pool(name="sb", bufs=1))
        sbx = ctx.enter_context(tc.tile_pool(name="sbx", bufs=2))
        sbr = ctx.enter_context(tc.tile_pool(name="sbr", bufs=6))
        sbe = ctx.enter_context(tc.tile_pool(name="sbe", bufs=4))
        sbm = ctx.enter_context(tc.tile_pool(name="sbm", bufs=8))
        sbg = ctx.enter_context(tc.tile_pool(name="sbg", bufs=4))
        ps = ctx.enter_context(tc.tile_pool(name="ps", bufs=4, space="PSUM"))
        dr = ctx.enter_context(tc.tile_pool(name="dr", bufs=4, space="DRAM"))

        nc.gpsimd.load_library(library_config.ap_gather)

        # loop-invariant operands (halved so the first matmul starts sooner)
        ema = sb.tile([128, K], BF16, tag="ema")
        emb = sb.tile([70, K], BF16, tag="emb")
        nc.sync.dma_start(ema[:, 0:512], d_ema[:, 0:512])
        nc.sync.dma_start(emb[:, 0:512], d_emb[:, 0:512])
        cbt = sb.tile([128, K], F32, tag="cbt")   # DMA deferred: first use ~50us in
        zeros = sb.tile([128, K], I16, tag="zeros")
        nc.vector.memset(zeros[:], 0)

        idx16 = [sb.tile([128, NTILE], I16, tag=f"idx16_{b}", name=f"idx16_{b}")
                 for b in range(BPC)]

        # output stage, split in two phases so the gpsimd ap_gather is only
        # emitted once its index staging DMAs have had a quarter to land
        # (avoids head-of-line blocking of the gpsimd scan queue)
        agx_tiles = {}

        def emit_half_stage(p, half):
            # stages 16 index columns (two quarters) for both batches of pair p
            HT2 = TOK // 2
            HG2 = NTILE // 2
            g0 = half * HG2
            agx = sbg.tile([128, HT2 // 16], I16, tag="agx",
                           name=f"agx_{p}_{half}")
            agx_tiles[(p, half)] = agx
            for h in range(2):
                st = dr.tile([128, HG2], I16, tag=f"st{h}",
                             name=f"st{h}_{p}_{half}")
                nc.sync.dma_start(st[:], idx16[2 * p + h][:, g0:g0 + HG2])
                src = st[:].rearrange("(b r) g -> r g b", b=8, r=16)
                for c in range(4):
                    q = 16 * (4 * h + c)
                    dst = agx[q:q + 16, :].rearrange(
                        "p (a b) -> p a b", a=HG2, b=8)
                    nc.sync.dma_start(dst, src)

        def emit_half_gather(p, half, chalf=None):
            # chalf None: gather all 2048 idxs; 0/1: 1024-idx halves
            HT2 = TOK // 2
            if chalf is None or chalf == 1:
                agx = agx_tiles.pop((p, half))
            else:
                agx = agx_tiles[(p, half)]
            if chalf is None:
                axv, nidx, off = agx[:], HT2, 0
            else:
                nidx = HT2 // 2
                axv = agx[:, 64 * chalf:64 * chalf + 64]
                off = chalf * nidx
            hs = slice(half * HT2 + off, half * HT2 + off + nidx)
            gout = sbg.tile([128, nidx], F32, tag="gout",
                            name=f"gout_{p}_{half}_{chalf}")
            nc.gpsimd.ap_gather(gout[:], cbt[:], axv,
                                channels=128, num_elems=K, d=1, num_idxs=nidx)
            nc.sync.dma_start(d_out[p][:, hs], gout[:])

        # deferred per-quarter output work: convert at +1 quarter, stage
        # right after, gather at +2 quarters (so nothing ever waits)
        idxf_of = {}
        accf_of = {}

        def quarter_convert(slot):
            b, q = slot // 4, slot % 4
            q8 = slice(q * 8, q * 8 + 8)
            if (slot * 5) % 16 < COUNT_ACT_16:
                # acc = 2k* - 1024  ->  k* = acc/2 + 512
                nc.vector.tensor_scalar(idx16[b][:, q8], accf_of[b][:, q8],
                                        0.5, 512.0,
                                        mybir.AluOpType.mult,
                                        mybir.AluOpType.add)
            else:
                nc.vector.tensor_scalar(idx16[b][:, q8], idxf_of[b][:, q8],
                                        1.0, None, mybir.AluOpType.mult)
            if b % 2 == 1 and q % 2 == 1:
                emit_half_stage(b // 2, q // 2)

        pending = []

        def emit_count(r, count_act, idxf, accf, b, g):
            # k* = #(r_k < m), m = r[:, K-1]
            if count_act:
                # ACT path: acc = sum_k sign(2(m - r_k) - 1) = 2k* - 1024
                # (argument is an odd integer, never 0)
                b2f = sbm.tile([128, 1], F32, tag="b2f", name=f"b2f_{b}_{g}")
                nc.vector.tensor_scalar(b2f[:], r[:, K - 1:K], 2.0, -1.0,
                                        mybir.AluOpType.mult,
                                        mybir.AluOpType.add)
                strash = sbe.tile([128, K], I16, tag="strash",
                                  name=f"st_{b}_{g}")
                nc.scalar.activation(strash[:], r[:],
                                     mybir.ActivationFunctionType.Sign,
                                     bias=b2f[:], scale=-2.0,
                                     accum_out=accf[:, g:g + 1])
            else:
                # DVE path: one 4x is_lt pass with count accumulate
                mf = sbm.tile([128, 1], F32, tag="mf", name=f"mf_{b}_{g}")
                nc.scalar.copy(mf[:], r[:, K - 1:K])
                eqt = sbe.tile([128, K], I16, tag="eqt", name=f"eq_{b}_{g}")
                nc.vector.tensor_scalar(eqt[:], r[:], mf[:], None,
                                        mybir.AluOpType.is_lt,
                                        mybir.AluOpType.add,
                                        accum_out=idxf[:, g:g + 1])

        tidx = 0
        for b in range(BPC):
            xma = sbx.tile([128, TOK], BF16, tag="xma", name=f"xma_{b}")
            xmb = sbx.tile([70, TOK], BF16, tag="xmb", name=f"xmb_{b}")
            chunks = ([(0, 256), (256, 1024)] if b == 0 else []) + \
                [(q0, q0 + TOK // 4) for q0 in range(TOK // 4 if b == 0 else 0,
                                                     TOK, TOK // 4)]
            for ci, (lo, hi) in enumerate(chunks):
                qs = slice(lo, hi)
                nc.sync.dma_start(xma[:, qs], d_xma[b][:, qs])
                nc.sync.dma_start(xmb[:, qs], d_xmb[b][:, qs])
                if b == 0 and ci == 0:
                    # second halves of the codebook operands follow the
                    # first token chunk so tile 0 can start ASAP
                    nc.sync.dma_start(ema[:, 512:K], d_ema[:, 512:K])
                    nc.sync.dma_start(emb[:, 512:K], d_emb[:, 512:K])
            if b == 0:
                nc.sync.dma_start(cbt[:], d_cbt[:])

            idxf = sbx.tile([128, NTILE], F32, tag="idxf", name=f"idxf_{b}")
            accf = sbx.tile([128, NTILE], F32, tag="accf", name=f"accf_{b}")
            idxf_of[b] = idxf
            accf_of[b] = accf

            for g in range(NTILE):
                gs = slice(g * 128, (g + 1) * 128)
                qidx = b * 4 + g // 8
                count_act = (qidx * 5) % 16 < COUNT_ACT_16
                pp = ps.tile([128, K], F32, tag="pp", name=f"pp_{b}_{g}")
                for ch in range(2):
                    cs = slice(ch * 512, (ch + 1) * 512)
                    nc.tensor.matmul(pp[:, cs], xma[:, gs], ema[:, cs],
                                     start=True, stop=False)
                    nc.tensor.matmul(pp[:, cs], xmb[:, gs], emb[:, cs],
                                     start=False, stop=True)

                # running max, quantized to i16 on downcast
                r = sbr.tile([128, K], I16, tag="r", name=f"r_{b}_{g}")
                use_gp = ((tidx * SCAN_GP_32) % 32 < SCAN_GP_32 and SCAN_GP_32 > 0
                          and tidx < BPC * NTILE - TAIL_DVE)
                seng = nc.gpsimd if use_gp else nc.vector
                seng.tensor_tensor_scan(r[:], pp[:], zeros[:], -32000.0,
                                        mybir.AluOpType.max, mybir.AluOpType.bypass)

                # defer the count by one tile so the DVE queue issues the
                # next scan before a count that waits on a gpsimd scan
                pending.append((r, count_act, idxf, accf, b, g))
                if len(pending) > 1:
                    emit_count(*pending.pop(0))
                tidx += 1
                if g % 8 == 7:
                    slot = b * 4 + g // 8     # this quarter just finished
                    if slot >= 1:
                        quarter_convert(slot - 1)
                    # gather whatever was staged at least one slot back
                    for key in [k for k in agx_tiles
                                if k[0] * 8 + 2 * k[1] + 7 <= slot]:
                        emit_half_gather(*key)
        # drain: final count + final convert + remaining stages/gathers;
        # the very last gather is split in two so its output DMA overlaps
        while pending:
            emit_count(*pending.pop(0))
        quarter_convert(BPC * 4 - 1)
        last = (BPC // 2 - 1, 1)
        for key in [k for k in sorted(agx_tiles) if k != last]:
            emit_half_gather(*key)
        emit_half_gather(*last, chalf=0)
        emit_half_gather(*last, chalf=1)

    nc.compile()
    return nc


def _prep_host(inputs, codebook):
    x = np.ascontiguousarray(inputs, dtype=np.float32)      # [B, 64, H, W]
    cb = np.ascontiguousarray(codebook, dtype=np.float32)

    xf = x.reshape(B, D, TOK)                               # channel-major tokens

    # A = ||flat||^2 with the reference's summation order
    flat = np.ascontiguousarray(x.transpose(0, 2, 3, 1)).reshape(-1, D)
    A = np.sum(flat * flat, axis=1).reshape(B, TOK)         # f32 [B, TOK]

    # per-token scale s = 1/ulp(T), T ~ A (b-c shifts only matter within
    # ~1e-2 of a power of 2; ~0 flips).  Clamp so |c|*s fits int16, and
    # clip the effective exponent to the device's bq indicator row range.
    e = np.floor(np.log2(A)).astype(np.int32)
    s = np.exp2(23 - e).astype(np.float32)
    emax = float(np.max(np.sqrt(np.sum(cb * cb, axis=1))))  # max_k ||e_k||
    cbound = 2.0 * np.sqrt(A) * emax                        # |c_tk| <= cbound_t
    smax = np.exp2(np.floor(np.log2(32700.0 / np.maximum(cbound, 1e-30))))
    s = np.minimum(s, smax)
    s = np.clip(s, np.exp2(23 - EMAX_ROW), np.exp2(23 - EMIN_ROW)).astype(np.float32)
    e_eff = (23.5 - np.log2(s)).astype(np.int32)            # in [EMIN_ROW, EMAX_ROW]

    s = s.reshape(B, TOK)
    e_eff = e_eff.reshape(B, TOK)
    xs = xf * s[:, None, :]                                 # exact (power of 2)
    xh, xl = _split2(xs)                                    # [B, 64, TOK] bf16

    e2 = (2.0 * cb).astype(np.float32)                      # exact
    e2h, e2l = _split2(e2.T)                                # [64, K] bf16

    ema = np.concatenate([e2h, e2l], axis=0)                # [128, K]
    # bq rows: T_k*s = A*s + rint(b_k*s) with A*s an exact integer, so the
    # per-k rounded-T shift is the small integer bq_e[k], selected per token
    # by an indicator row.  HALF folds in the trunc-vs-RNE knob.
    bnorm = np.sum(cb.astype(np.float64) * cb.astype(np.float64), axis=1)
    emb = np.zeros((70, K), ml_dtypes.bfloat16)
    emb[0:64] = e2h
    for j, ee in enumerate(range(EMIN_ROW, EMAX_ROW + 1)):
        emb[64 + j] = (HALF - np.rint(bnorm * np.exp2(23 - ee))).astype(ml_dtypes.bfloat16)

    cbt = np.ascontiguousarray(cb.T)                        # [64, K]
    cbt_d = np.concatenate([cbt, cbt], axis=0)              # [128, K]

    in_maps = []
    for cid in range(NCORES):
        b0 = BPC * cid
        xma = np.empty((BPC, 128, TOK), ml_dtypes.bfloat16)
        xmb = np.empty((BPC, 70, TOK), ml_dtypes.bfloat16)
        for j in range(BPC):
            xma[j, 0:64] = xl[b0 + j]
            xma[j, 64:128] = xh[b0 + j]
            xmb[j, 0:64] = xh[b0 + j]
            for i, ee in enumerate(range(EMIN_ROW, EMAX_ROW + 1)):
                xmb[j, 64 + i] = (e_eff[b0 + j] == ee).astype(ml_dtypes.bfloat16)
        in_maps.append({
            "xma": xma, "xmb": xmb,
            "ema": ema, "emb": emb,
            "cbt": cbt_d,
        })
    return in_maps


def _run(inputs, codebook, trace=False):
    if "nc" not in _cache:
        _cache["nc"] = _build_module()
    nc = _cache["nc"]
    in_maps = _prep_host(inputs, codebook)
    res = bass_utils.run_bass_kernel_spmd(
        nc, in_maps, core_ids=list(range(NCORES)), trace=trace)
    outs = np.empty((B, D, H, W), np.float32)
    for cid in range(NCORES):
        o = res.results[cid]["out"]              # [2, 128, TOK]
        outs[BPC * cid: BPC * (cid + 1)] = o.reshape(BPC, D, H, W)
    return outs, res


def kernel(inputs, codebook):
    out, _ = _run(inputs, codebook, trace=False)
    return out



# revision 32
# speedup vs baseline: 1.0344x; 1.0344x over previous
"""VQ codebook-lookup kernel v2 for trn2 (8 NeuronCores, SPMD data-parallel).

For x: [32, 64, 64, 64] (BCHW) and codebook: [1024, 64]:
    out[b,:,h,w] = codebook[argmin_k fl(fl(A+b_k) - c_k)],  c_k = fl(2 flat.e_k)
matching the f32 reference's rounding/tie structure.

Key algebraic reduction vs the old kernel: since fl is monotone,
    argmin_k fl(T_t - c_k)  ==  first k maximizing round(c_k / ulp(T_t))
so instead of reproducing fl(A+b) on the PE (6-row chain, 4 matmul passes),
the host pre-scales x by s_t = 1/ulp(T_t) (a power of 2, exact) and the PE
computes y = c*s + 8192(.5) in TWO bf16-split passes:
    m_a = [xl;xh] . [e2h;e2l]        (contraction 128: xl*e2h + xh*e2l)
    m_b = [xh;1;1] . [e2h;8192;HALF] (contraction 66: xh*e2h + bias)
A tensor_tensor_scan(max) reads the PSUM, tracks the running max in f32,
and its int16 downcast quantizes to the reference's rounding grid.  The
first-argmax index (reference tie-break) is the count of (r_k < m), one
4x-mode DVE tensor_scalar(is_lt, accum=add).  Scans are split DVE/gpsimd.

Sharding: batch-parallel, core i handles batches [4i, 4i+4).
Output = gathered codebook rows (STE forward value == q up to ~2e-4 rel).
"""

import sys
import numpy as np
import ml_dtypes
from contextlib import ExitStack

for p in ("/opt/trn_rl_repo",):
    if p not in sys.path:
        sys.path.append(p)

import concourse.bacc as bacc
import concourse.mybir as mybir
import concourse.tile as tile
from concourse import bass_utils, library_config

F32 = mybir.dt.float32
BF16 = mybir.dt.bfloat16
I16 = mybir.dt.int16

B, D, H, W = 32, 64, 64, 64
K = 1024
NCORES = 8
BPC = B // NCORES          # batches per core = 4
TOK = H * W                # tokens per batch = 4096
NTILE = TOK // 128         # 128-token tiles per batch = 32

# knobs
HALF = 0.0                 # 0.5 if the f32->i16 downcast truncates, 0.0 if RNE
EMIN_ROW, EMAX_ROW = 4, 9  # bq indicator row exponent range (6 rows)
SCAN_GP_32 = 18            # tile's scan runs on gpsimd when (tidx*this)%32 < this
COUNT_ACT_16 = 4           # q8-group's counts run on ACT when (qidx % 16) < this
TAIL_DVE = 0               # last N tiles force DVE scans (shorter drain)
PENDING_DEPTH = 1          # count deferral depth in tiles

_cache = {}


def _bf16(v):
    return v.astype(ml_dtypes.bfloat16)


def _split2(v):
    h = _bf16(v)
    l = _bf16(v - h.astype(np.float32))
    return h, l


def _build_module():
    nc = bacc.Bacc("TRN2", target_bir_lowering=False, debug=False, num_devices=NCORES)

    d_xma = nc.dram_tensor("xma", [BPC, 128, TOK], BF16, kind="ExternalInput").ap()
    d_xmb = nc.dram_tensor("xmb", [BPC, 70, TOK], BF16, kind="ExternalInput").ap()
    d_ema = nc.dram_tensor("ema", [128, K], BF16, kind="ExternalInput").ap()
    d_emb = nc.dram_tensor("emb", [70, K], BF16, kind="ExternalInput").ap()
    d_cbt = nc.dram_tensor("cbt", [128, K], F32, kind="ExternalInput").ap()
    d_out = nc.dram_tensor("out", [2, 128, TOK], F32, kind="ExternalOutput").ap()

    with tile.TileContext(nc) as tc, ExitStack() as ctx:
        sb = ctx.enter_context(tc.tile_pool(name="sb", bufs=1))
        sbx = ctx.enter_context(tc.tile_pool(name="sbx", bufs=2))
        sbr = ctx.enter_context(tc.tile_pool(name="sbr", bufs=6))
        sbe = ctx.enter_context(tc.tile_pool(name="sbe", bufs=4))
        sbm = ctx.enter_context(tc.tile_pool(name="sbm", bufs=8))
        sbg = ctx.enter_context(tc.tile_pool(name="sbg", bufs=4))
        ps = ctx.enter_context(tc.tile_pool(name="ps", bufs=4, space="PSUM"))
        dr = ctx.enter_context(tc.tile_pool(name="dr", bufs=4, space="DRAM"))

        nc.gpsimd.load_library(library_config.ap_gather)

        # loop-invariant operands (halved so the first matmul starts sooner)
        ema = sb.tile([128, K], BF16, tag="ema")
        emb = sb.tile([70, K], BF16, tag="emb")
        nc.sync.dma_start(ema[:, 0:512], d_ema[:, 0:512])
        nc.sync.dma_start(emb[:, 0:512], d_emb[:, 0:512])
        cbt = sb.tile([128, K], F32, tag="cbt")   # DMA deferred: first use ~50us in
        zeros = sb.tile([128, K], I16, tag="zeros")
        nc.vector.memset(zeros[:], 0)

        idx16 = [sb.tile([128, NTILE], I16, tag=f"idx16_{b}", name=f"idx16_{b}")
                 for b in range(BPC)]

        # output stage, split in two phases so the gpsimd ap_gather is only
        # emitted once its index staging DMAs have had a quarter to land
        # (avoids head-of-line blocking of the gpsimd scan queue)
        agx_tiles = {}

        def emit_half_stage(p, half):
            HT = TOK // 4
            HG = NTILE // 4
            g0 = half * HG
            agx = sbg.tile([128, HT // 16], I16, tag="agx",
                           name=f"agx_{p}_{half}")
            agx_tiles[(p, half)] = agx
            for h in range(2):
                st = dr.tile([128, HG], I16, tag=f"st{h}",
                             name=f"st{h}_{p}_{half}")
                nc.sync.dma_start(st[:], idx16[2 * p + h][:, g0:g0 + HG])
                src = st[:].rearrange("(b r) g -> r g b", b=8, r=16)
                for c in range(4):
                    q = 16 * (4 * h + c)
                    dst = agx[q:q + 16, :].rearrange(
                        "p (a b) -> p a b", a=HG, b=8)
                    nc.sync.dma_start(dst, src)

        def emit_half_gather(p, half):
            HT = TOK // 4
            agx = agx_tiles.pop((p, half))
            hs = slice(half * HT, (half + 1) * HT)
            gout = sbg.tile([128, HT], F32, tag="gout",
                            name=f"gout_{p}_{half}")
            nc.gpsimd.ap_gather(gout[:], cbt[:], agx[:],
                                channels=128, num_elems=K, d=1, num_idxs=HT)
            nc.sync.dma_start(d_out[p][:, hs], gout[:])

        # deferred per-quarter output work: convert at +1 quarter, stage
        # right after, gather at +2 quarters (so nothing ever waits)
        idxf_of = {}
        accf_of = {}

        def quarter_convert(slot):
            b, q = slot // 4, slot % 4
            q8 = slice(q * 8, q * 8 + 8)
            if (slot * 5) % 16 < COUNT_ACT_16:
                # acc = 2k* - 1024  ->  k* = acc/2 + 512
                nc.vector.tensor_scalar(idx16[b][:, q8], accf_of[b][:, q8],
                                        0.5, 512.0,
                                        mybir.AluOpType.mult,
                                        mybir.AluOpType.add)
            else:
                nc.vector.tensor_scalar(idx16[b][:, q8], idxf_of[b][:, q8],
                                        1.0, None, mybir.AluOpType.mult)
            if b % 2 == 1:
                emit_half_stage(b // 2, q)

        pending = []

        def emit_count(r, count_act, idxf, accf, b, g):
            # k* = #(r_k < m), m = r[:, K-1]
            if count_act:
                # ACT path: acc = sum_k sign(2(m - r_k) - 1) = 2k* - 1024
                # (argument is an odd integer, never 0)
                b2f = sbm.tile([128, 1], F32, tag="b2f", name=f"b2f_{b}_{g}")
                nc.vector.tensor_scalar(b2f[:], r[:, K - 1:K], 2.0, -1.0,
                                        mybir.AluOpType.mult,
                                        mybir.AluOpType.add)
                strash = sbe.tile([128, K], I16, tag="strash",
                                  name=f"st_{b}_{g}")
                nc.scalar.activation(strash[:], r[:],
                                     mybir.ActivationFunctionType.Sign,
                                     bias=b2f[:], scale=-2.0,
                                     accum_out=accf[:, g:g + 1])
            else:
                # DVE path: one 4x is_lt pass with count accumulate
                mf = sbm.tile([128, 1], F32, tag="mf", name=f"mf_{b}_{g}")
                nc.scalar.copy(mf[:], r[:, K - 1:K])
                eqt = sbe.tile([128, K], I16, tag="eqt", name=f"eq_{b}_{g}")
                nc.vector.tensor_scalar(eqt[:], r[:], mf[:], None,
                                        mybir.AluOpType.is_lt,
                                        mybir.AluOpType.add,
                                        accum_out=idxf[:, g:g + 1])

        tidx = 0
        for b in range(BPC):
            xma = sbx.tile([128, TOK], BF16, tag="xma", name=f"xma_{b}")
            xmb = sbx.tile([70, TOK], BF16, tag="xmb", name=f"xmb_{b}")
            chunks = ([(0, 256), (256, 1024)] if b == 0 else []) + \
                [(q0, q0 + TOK // 4) for q0 in range(TOK // 4 if b == 0 else 0,
                                                     TOK, TOK // 4)]
            for ci, (lo, hi) in enumerate(chunks):
                qs = slice(lo, hi)
                nc.sync.dma_start(xma[:, qs], d_xma[b][:, qs])
                nc.sync.dma_start(xmb[:, qs], d_xmb[b][:, qs])
                if b == 0 and ci == 0:
                    # second halves of the codebook operands follow the
                    # first token chunk so tile 0 can start ASAP
                    nc.sync.dma_start(ema[:, 512:K], d_ema[:, 512:K])
                    nc.sync.dma_start(emb[:, 512:K], d_emb[:, 512:K])
            if b == 0:
                nc.sync.dma_start(cbt[:], d_cbt[:])

            idxf = sbx.tile([128, NTILE], F32, tag="idxf", name=f"idxf_{b}")
            accf = sbx.tile([128, NTILE], F32, tag="accf", name=f"accf_{b}")
            idxf_of[b] = idxf
            accf_of[b] = accf

            for g in range(NTILE):
                gs = slice(g * 128, (g + 1) * 128)
                qidx = b * 4 + g // 8
                count_act = (qidx * 5) % 16 < COUNT_ACT_16
                pp = ps.tile([128, K], F32, tag="pp", name=f"pp_{b}_{g}")
                for ch in range(2):
                    cs = slice(ch * 512, (ch + 1) * 512)
                    nc.tensor.matmul(pp[:, cs], xma[:, gs], ema[:, cs],
                                     start=True, stop=False)
                    nc.tensor.matmul(pp[:, cs], xmb[:, gs], emb[:, cs],
                                     start=False, stop=True)

                # running max, quantized to i16 on downcast
                r = sbr.tile([128, K], I16, tag="r", name=f"r_{b}_{g}")
                use_gp = ((tidx * SCAN_GP_32) % 32 < SCAN_GP_32 and SCAN_GP_32 > 0
                          and tidx < BPC * NTILE - TAIL_DVE)
                seng = nc.gpsimd if use_gp else nc.vector
                seng.tensor_tensor_scan(r[:], pp[:], zeros[:], -32000.0,
                                        mybir.AluOpType.max, mybir.AluOpType.bypass)

                # defer the count by one tile so the DVE queue issues the
                # next scan before a count that waits on a gpsimd scan
                pending.append((r, count_act, idxf, accf, b, g))
                if len(pending) > PENDING_DEPTH:
                    emit_count(*pending.pop(0))
                tidx += 1
                if g % 8 == 7:
                    slot = b * 4 + g // 8     # this quarter just finished
                    if slot >= 1:
                        quarter_convert(slot - 1)
                    # gather whatever was staged two slots back
                    for key in [k for k in agx_tiles
                                if k[0] * 8 + 4 + k[1] <= slot - 2]:
                        emit_half_gather(*key)
        # drain: final count + final convert + remaining stages/gathers
        while pending:
            emit_count(*pending.pop(0))
        quarter_convert(BPC * 4 - 1)
        for key in sorted(agx_tiles):
            emit_half_gather(*key)

    nc.compile()
    return nc


def _prep_host(inputs, codebook):
    x = np.ascontiguousarray(inputs, dtype=np.float32)      # [B, 64, H, W]
    cb = np.ascontiguousarray(codebook, dtype=np.float32)

    xf = x.reshape(B, D, TOK)                               # channel-major tokens

    # A = ||flat||^2 with the reference's summation order
    flat = np.ascontiguousarray(x.transpose(0, 2, 3, 1)).reshape(-1, D)
    A = np.sum(flat * flat, axis=1).reshape(B, TOK)         # f32 [B, TOK]

    # per-token scale s = 1/ulp(T), T ~ A (b-c shifts only matter within
    # ~1e-2 of a power of 2; ~0 flips).  Clamp so |c|*s fits int16, and
    # clip the effective exponent to the device's bq indicator row range.
    e = np.floor(np.log2(A)).astype(np.int32)
    s = np.exp2(23 - e).astype(np.float32)
    emax = float(np.max(np.sqrt(np.sum(cb * cb, axis=1))))  # max_k ||e_k||
    cbound = 2.0 * np.sqrt(A) * emax                        # |c_tk| <= cbound_t
    smax = np.exp2(np.floor(np.log2(32700.0 / np.maximum(cbound, 1e-30))))
    s = np.minimum(s, smax)
    s = np.clip(s, np.exp2(23 - EMAX_ROW), np.exp2(23 - EMIN_ROW)).astype(np.float32)
    e_eff = (23.5 - np.log2(s)).astype(np.int32)            # in [EMIN_ROW, EMAX_ROW]

    s = s.reshape(B, TOK)
    e_eff = e_eff.reshape(B, TOK)
    xs = xf * s[:, None, :]                                 # exact (power of 2)
    xh, xl = _split2(xs)                                    # [B, 64, TOK] bf16

    e2 = (2.0 * cb).astype(np.float32)                      # exact
    e2h, e2l = _split2(e2.T)                                # [64, K] bf16

    ema = np.concatenate([e2h, e2l], axis=0)                # [128, K]
    # bq rows: T_k*s = A*s + rint(b_k*s) with A*s an exact integer, so the
    # per-k rounded-T shift is the small integer bq_e[k], selected per token
    # by an indicator row.  HALF folds in the trunc-vs-RNE knob.
    bnorm = np.sum(cb.astype(np.float64) * cb.astype(np.float64), axis=1)
    emb = np.zeros((70, K), ml_dtypes.bfloat16)
    emb[0:64] = e2h
    for j, ee in enumerate(range(EMIN_ROW, EMAX_ROW + 1)):
        emb[64 + j] = (HALF - np.rint(bnorm * np.exp2(23 - ee))).astype(ml_dtypes.bfloat16)

    cbt = np.ascontiguousarray(cb.T)                        # [64, K]
    cbt_d = np.concatenate([cbt, cbt], axis=0)              # [128, K]

    in_maps = []
    for cid in range(NCORES):
        b0 = BPC * cid
        xma = np.empty((BPC, 128, TOK), ml_dtypes.bfloat16)
        xmb = np.empty((BPC, 70, TOK), ml_dtypes.bfloat16)
        for j in range(BPC):
            xma[j, 0:64] = xl[b0 + j]
            xma[j, 64:128] = xh[b0 + j]
            xmb[j, 0:64] = xh[b0 + j]
            for i, ee in enumerate(range(EMIN_ROW, EMAX_ROW + 1)):
                xmb[j, 64 + i] = (e_eff[b0 + j] == ee).astype(ml_dtypes.bfloat16)
        in_maps.append({
            "xma": xma, "xmb": xmb,
            "ema": ema, "emb": emb,
            "cbt": cbt_d,
        })
    return in_maps


def _run(inputs, codebook, trace=False):
    if "nc" not in _cache:
        _cache["nc"] = _build_module()
    nc = _cache["nc"]
    in_maps = _prep_host(inputs, codebook)
    res = bass_utils.run_bass_kernel_spmd(
        nc, in_maps, core_ids=list(range(NCORES)), trace=trace)
    outs = np.empty((B, D, H, W), np.float32)
    for cid in range(NCORES):
        o = res.results[cid]["out"]              # [2, 128, TOK]
        outs[BPC * cid: BPC * (cid + 1)] = o.reshape(BPC, D, H, W)
    return outs, res


def kernel(inputs, codebook):
    out, _ = _run(inputs, codebook, trace=False)
    return out

